# revision 19
# baseline (speedup 1.0000x reference)
"""Multi-head attention (B=2, S=2048, E=1024, H=16, causal) on 8 Trainium2 cores.

Sharding: data-parallel over batch (2) x tensor-parallel over heads (4 groups
of 4 heads). Core i handles batch i//4, heads 4*(i%4) .. 4*(i%4)+3.
Each core computes Q/K/V projections for its 256 channels, causal
flash-attention for its 4 heads, and a partial output projection
(contribution of its channels to all 1024 output features). Partials are
summed across the 4 cores of each batch group (host-side).

Key optimizations over the straightforward version:
- host-side DRAM layouts match SBUF destinations exactly: every input DMA
  is 128 contiguous descriptors (cuts the descriptor-bound startup)
- diagonal-tile column restriction: scores/exp/mask/PV only touch the
  causally-valid q columns of diagonal k-tiles (ideal-causal PE/ACT work)
- PV matmuls deferred across head-pair boundaries: the next pair's score
  matmuls keep the PE fed while this pair's last exps drain on ACT
- merged output projection; tail stores alternate sync/scalar queues
"""
import numpy as np

import concourse.bass as bass
import concourse.tile as tile
from concourse import bacc, mybir
from concourse.bass_utils import run_bass_kernel_spmd

F32 = mybir.dt.float32
F32R = mybir.dt.float32r
BF16 = mybir.dt.bfloat16
import ml_dtypes
MM_DT = BF16
MM_NP = ml_dtypes.bfloat16
OUT_DT = BF16
ActF = mybir.ActivationFunctionType
Alu = mybir.AluOpType

B, S, E = 2, 2048, 1024
H, DH = 16, 64
NCORES, TPW = 8, 4          # 8 cores, 4-way tensor parallel per batch
HPC = H // TPW              # heads per core = 4
C = HPC * DH                # channels per core = 256
SCALE = 1.0 / 8.0           # 1/sqrt(DH)
VW = HPC * (DH + 1)         # V storage width per s-tile (ones col per head)
NST = S // 128              # 16 s-tiles of 128 rows
NQB = S // 512              # 4 q-blocks of 512
NEC = E // 128              # 8 e-chunks (contraction for projections)
CST_P = 4 + NST * HPC       # cstp cols: bqk | ones (V)

_cache = {}


def _emit(nc, tc, causal):
    # ---- DRAM parameters (layouts match SBUF destinations: contiguous DMA) ----
    xt_d = nc.dram_tensor("xt", [128, NQB * NEC * 512], MM_DT,
                          kind="ExternalInput").ap()
    wq_d = [nc.dram_tensor(f"wq{t}", [128, NEC * 128], MM_DT,
                           kind="ExternalInput").ap() for t in range(2)]
    wk_d = [nc.dram_tensor(f"wk{t}", [128, NEC * 128], MM_DT,
                           kind="ExternalInput").ap() for t in range(2)]
    wvt_d = nc.dram_tensor("wvt", [128, NEC * C], MM_DT, kind="ExternalInput").ap()
    wot_d = nc.dram_tensor("wot", [128, 2 * E], MM_DT, kind="ExternalInput").ap()
    cstp_d = nc.dram_tensor("cstp", [128, CST_P], F32, kind="ExternalInput").ap()
    cstr_d = nc.dram_tensor("cstr", [1, C + E + 128], F32,
                            kind="ExternalInput").ap()
    out_d = nc.dram_tensor("out", [S, E], OUT_DT, kind="ExternalOutput").ap()

    ctxpool = tc.tile_pool

    with ctxpool(name="persist", bufs=1) as pp:
        # ---- persistent SBUF tensors ----
        # X^T layout [p, sb, ec, c]: matches DRAM exactly.
        xt_sb = pp.tile([128, NEC * S], MM_DT)

        def xap(ec, s0, w):
            sb, o = divmod(s0, 512)
            c0 = sb * (NEC * 512) + ec * 512 + o
            return xt_sb[:, c0:c0 + w]

        wvt_sb = pp.tile([128, NEC * C], MM_DT)
        wot_sb = pp.tile([128, 2 * E], MM_DT)        # c-chunk cc at cols [cc*E)
        qt_sb = pp.tile([128, 2 * S], MM_DT)         # Q^T, d-tile t at cols [t*S)
        kt_sb = pp.tile([128, 2 * S], MM_DT)
        v_sb = pp.tile([128, NST * VW], MM_DT)       # V (+ones col per head)
        ot_sb = pp.tile([128, 2 * S], MM_DT)         # normalized attn out^T
        cstp_sb = pp.tile([128, CST_P], F32)
        bvb_sb = pp.tile([128, C], F32)             # bv broadcast to partitions
        bob_sb = pp.tile([128, E], F32)             # bo broadcast to partitions
        bqk_sb = cstp_sb[:, 0:4]
        onesb_sb = cstp_sb[:, 4:4 + NST * HPC]

        def emit_vproj(psum_pool, st, vtag="mps"):
            """Project V for s-tile st into v_sb (with per-head ones column)."""
            ps = psum_pool.tile([128, C], F32, tag=vtag, name=f"vp{st}")
            for ec in range(NEC):
                nc.tensor.matmul(
                    ps[:],
                    xap(ec, st * 128, 128),
                    wvt_sb[:, ec * C: (ec + 1) * C],
                    start=(ec == 0), stop=(ec == NEC - 1),
                    skip_group_check=True)
            dst = v_sb[:, st * VW: st * VW + VW].rearrange(
                "p (h x) -> p h x", h=HPC)[:, :, 0:DH]
            nc.vector.tensor_add(
                dst,
                ps[:].rearrange("p (h x) -> p h x", h=HPC),
                bvb_sb[:].rearrange("p (h x) -> p h x", h=HPC))

        with ctxpool(name="qkw", bufs=1) as qkw, \
             ctxpool(name="small", bufs=1) as sp:
            wq_sb = [qkw.tile([128, NEC * 128], MM_DT, name=f"wq{t}_sb")
                     for t in range(2)]
            wk_sb = [qkw.tile([128, NEC * 128], MM_DT, name=f"wk{t}_sb")
                     for t in range(2)]
            cstr_sb = sp.tile([1, C + E + 128], F32R)
            bv_row = cstr_sb[0:1, 0:C]
            bo_row = cstr_sb[0:1, C:C + E]
            ones_r = cstr_sb[0:1, C + E:C + E + 128]

            # ---- input DMAs, split across the two HWDGE queues in
            # consumption order; 0.25MB x chunks so the first projection
            # groups start mid-transfer ----
            def x_load(sb_i):
                for ep in range(NEC // 2):
                    eng = nc.sync if ep % 2 == 0 else nc.scalar
                    c0 = sb_i * (NEC * 512) + 2 * ep * 512
                    eng.dma_start(out=xt_sb[:, c0:c0 + 1024],
                                  in_=xt_d[:, c0:c0 + 1024])

            nc.scalar.dma_start(out=cstp_sb[:], in_=cstp_d[:])
            nc.scalar.dma_start(out=cstr_sb[:], in_=cstr_d[:].bitcast(F32R))
            nc.sync.dma_start(out=wq_sb[0][:], in_=wq_d[0][:])
            nc.scalar.dma_start(out=wk_sb[0][:], in_=wk_d[0][:])
            x_load(0)
            nc.sync.dma_start(out=wvt_sb[:], in_=wvt_d[:])
            x_load(1)
            x_load(2)
            nc.sync.dma_start(out=wq_sb[1][:], in_=wq_d[1][:])
            nc.scalar.dma_start(out=wk_sb[1][:], in_=wk_d[1][:])
            x_load(3)
            nc.scalar.dma_start(out=wot_sb[:], in_=wot_d[:])

            # ==== phase B: Q^T/K^T projections (s-block outer) ====
            with ctxpool(name="proj_ps", bufs=4, space="PSUM") as proj_ps:
                for sb_i in range(NQB):
                    for pj, w_sb, o_sb, bcol in ((0, wq_sb[0], qt_sb, 0),
                                                 (1, wk_sb[0], kt_sb, 2)):
                        ps = proj_ps.tile([128, 512], F32, tag="pps",
                                          name=f"pp0_{pj}_{sb_i}")
                        for ec in range(NEC):
                            nc.tensor.matmul(
                                ps[:],
                                w_sb[:, ec * 128: ec * 128 + 128],
                                xap(ec, sb_i * 512, 512),
                                start=(ec == 0), stop=(ec == NEC - 1),
                                skip_group_check=True)
                        nc.vector.tensor_scalar_add(
                            o_sb[:, sb_i * 512: sb_i * 512 + 512],
                            ps[:], bqk_sb[:, bcol: bcol + 1])
                    if sb_i == 1:
                        # bv broadcast, then V for the first q-block (all of
                        # V when non-causal)
                        ps_bv = proj_ps.tile([128, C], F32, tag="pps")
                        nc.tensor.matmul(ps_bv[:], ones_r[0:1, 0:128],
                                         bv_row[:], start=True, stop=True)
                        nc.vector.tensor_copy(bvb_sb[:], ps_bv[:])
                        for st in range(4):
                            emit_vproj(proj_ps, st, vtag="pps")
                        # V ones columns via a strided DVE copy
                        v_ones_ap = v_sb[:].rearrange(
                            "p (n x) -> p n x", x=DH + 1)[:, :, DH:DH + 1]
                        nc.vector.tensor_copy(
                            v_ones_ap,
                            onesb_sb[:].rearrange("p (n x) -> p n x", x=1))
                    if sb_i == 2:
                        for eb in range(2):
                            ps_bo = proj_ps.tile([128, 512], F32, tag="pps",
                                                 name=f"bo{eb}")
                            nc.tensor.matmul(
                                ps_bo[:], ones_r[0:1, 0:128],
                                bo_row[0:1, eb * 512:(eb + 1) * 512],
                                start=True, stop=True)
                            nc.vector.tensor_copy(
                                bob_sb[:, eb * 512:(eb + 1) * 512], ps_bo[:])
                if not causal:
                    for st in range(4, NST):
                        emit_vproj(proj_ps, st, vtag="pps")

            # ==== phase C: attention (q-block outer, head inner) + out-proj ====
            with ctxpool(name="score_ps", bufs=2, space="PSUM") as score_ps, \
                 ctxpool(name="attn_ps", bufs=2, space="PSUM") as attn_ps, \
                 ctxpool(name="misc_ps", bufs=2, space="PSUM") as misc_ps, \
                 ctxpool(name="pt_pool", bufs=10) as pt_pool, \
                 ctxpool(name="rec_pool", bufs=4) as rec_pool, \
                 ctxpool(name="out_pool", bufs=8) as out_pool:
                pending = []    # deferred norm closures of the previous hp
                pending_f = []  # deferred filler closures: (tag, closure)

                def flush_pending():
                    while pending:
                        pending.pop(0)()

                def flush_one():
                    if pending_f:
                        pending_f.pop(0)[1]()

                def flush_fillers(k=None):
                    n = len(pending_f) if k is None else min(k, len(pending_f))
                    for _ in range(n):
                        flush_one()

                def emit_dt1_part(sb_i, pj):
                    # one second-d-tile Q or K projection block (spread as
                    # fillers across the early attention steps)
                    w_sb, o_sb, bcol = ((wq_sb[1], qt_sb, 0),
                                        (wk_sb[1], kt_sb, 2))[pj]
                    with tc.high_priority(offset=-1_000_000):
                        ps1 = misc_ps.tile([128, 512], F32, tag="mps",
                                           name=f"pp1_{pj}_{sb_i}")
                        for ec in range(NEC):
                            nc.tensor.matmul(
                                ps1[:],
                                w_sb[:, ec * 128: ec * 128 + 128],
                                xap(ec, sb_i * 512, 512),
                                start=(ec == 0), stop=(ec == NEC - 1),
                                skip_group_check=True)
                        nc.vector.tensor_scalar_add(
                            o_sb[:, S + sb_i * 512: S + sb_i * 512 + 512],
                            ps1[:], bqk_sb[:, bcol + 1: bcol + 2])

                ot_half = {}
                # last-4 stores alternate queues so the tail drains 2x faster
                store_eng = {12: nc.scalar, 13: nc.sync,
                             14: nc.scalar, 15: nc.sync}

                def emit_outproj_st(qb, st, mode="full", last=False):
                    # out-projection for s-tile st. mode "full": both c-chunks
                    # accumulated in PSUM; "cc0"/"cc1": the two head-pair
                    # halves split so the last q-block's cc0 half can serve as
                    # PE filler during its hp1 attention steps.
                    with tc.high_priority(offset=0 if last else -1_000_000):
                        if mode == "cc1":
                            o_t = ot_half[st]
                        else:
                            o_t = out_pool.tile([128, E], OUT_DT, tag="ob",
                                                name=f"ot{st}")
                            ot_half[st] = o_t
                        for eb in range(2):
                            ps_f = misc_ps.tile([128, 512], F32, tag="mps",
                                                name=f"pg{st}{eb}{mode}")
                            if mode != "cc1":
                                nc.tensor.matmul(
                                    ps_f[:],
                                    ot_sb[:, st * 128: st * 128 + 128],
                                    wot_sb[:, eb * 512: eb * 512 + 512],
                                    start=True, stop=(mode == "cc0"),
                                    skip_group_check=True)
                            if mode != "cc0":
                                nc.tensor.matmul(
                                    ps_f[:],
                                    ot_sb[:, S + st * 128: S + st * 128 + 128],
                                    wot_sb[:, E + eb * 512: E + eb * 512 + 512],
                                    start=(mode == "cc1"), stop=True,
                                    skip_group_check=True)
                            if mode == "cc1":
                                nc.vector.tensor_add(
                                    o_t[:, eb * 512:(eb + 1) * 512], ps_f[:],
                                    o_t[:, eb * 512:(eb + 1) * 512])
                            else:
                                nc.vector.tensor_add(
                                    o_t[:, eb * 512:(eb + 1) * 512], ps_f[:],
                                    bob_sb[:, eb * 512:(eb + 1) * 512])
                        if mode != "cc0":
                            store_eng.get(st, nc.sync).dma_start(
                                out=out_d[st * 128:(st + 1) * 128, :],
                                in_=o_t[:])

                def emit_vproj_filler(st):
                    with tc.high_priority(offset=-1_000_000):
                        emit_vproj(misc_ps, st)

                def emit_outproj_cc1_tail(st):
                    # tail half of the split out-projection: one 1024-wide
                    # matmul into the (then idle) score PSUM pool and a single
                    # merged DVE add, so consecutive s-tiles pipeline through
                    # two PSUM generations instead of serializing on misc_ps
                    o_t = ot_half[st]
                    ps_f = score_ps.tile([128, 1024], F32, tag="sc",
                                         name=f"pgt{st}")
                    for eb in range(2):
                        nc.tensor.matmul(
                            ps_f[:, eb * 512:(eb + 1) * 512],
                            ot_sb[:, S + st * 128: S + st * 128 + 128],
                            wot_sb[:, E + eb * 512: E + (eb + 1) * 512],
                            start=True, stop=True,
                            skip_group_check=True)
                    nc.vector.tensor_add(o_t[:], ps_f[:], o_t[:])
                    store_eng.get(st, nc.sync).dma_start(
                        out=out_d[st * 128:(st + 1) * 128, :],
                        in_=o_t[:])

                pv_queue = []   # deferred PV closures, kept across hp bounds

                def pop_pv(limit):
                    while len(pv_queue) > limit:
                        pv_queue.pop(0)()

                for qb in range(NQB):
                    nk = 4 * (qb + 1) if causal else NST
                    q0 = qb * 512
                    if qb == 0:
                        # second-d-tile projections drip-fed as fillers
                        for sb_i in range(NQB):
                            for pj in range(2):
                                pending_f.append(
                                    (("dt1", sb_i),
                                     lambda sb_i=sb_i, pj=pj:
                                     emit_dt1_part(sb_i, pj)))
                    for hp in range(2):   # head pair (2*hp, 2*hp+1), d-tile hp
                        t = hp
                        ps_os = [None, None]
                        if hp == 1:
                            # hp1 scores need the d-tile-1 Q/K of every
                            # s-block this q-block touches
                            need = (nk - 1) // 4
                            while any(tg[0] == "dt1" and tg[1] <= need
                                      for tg, _ in pending_f):
                                flush_one()

                        def norm(qb=qb, hp=hp, t=t, q0=q0, ps_os=ps_os,
                                 lo=0, hi=512, chunks=None):
                            w = hi - lo
                            for a in range(2):
                                h = 2 * hp + a
                                p0 = a * 64
                                rs = rec_pool.tile([1, w], F32R, tag="rs",
                                                   name=f"rs{qb}{h}{lo}")
                                nc.vector.tensor_copy(rs[:],
                                                      ps_os[a][64:65, lo:hi])
                                ps_b = misc_ps.tile([64, w], F32, tag="mps",
                                                    name=f"pb{qb}{h}{lo}")
                                nc.tensor.matmul(ps_b[:], ones_r[0:1, 0:64],
                                                 rs[:], start=True, stop=True)
                                bc = rec_pool.tile([64, w], F32, tag="bc",
                                                   name=f"bc{qb}{h}{lo}")
                                nc.vector.reciprocal_approx_fast(bc[:], ps_b[:])
                                for c0, c1 in (chunks or ((lo, hi),)):
                                    nc.vector.tensor_mul(
                                        ot_sb[p0:p0 + 64,
                                              t * S + q0 + c0: t * S + q0 + c1],
                                        ps_os[a][0:64, c0:c1],
                                        bc[:, c0 - lo:c1 - lo])

                        def emit_pv(kt_i, pt, col0, hp=hp, nk=nk,
                                    ps_os=ps_os, qb=qb):
                            if causal:
                                # the V tile for this k-step may still be a
                                # queued filler
                                while any(tg == ("vp", kt_i)
                                          for tg, _ in pending_f):
                                    flush_one()
                            if kt_i == 0:
                                for a in range(2):
                                    ps_os[a] = attn_ps.tile(
                                        [65, 512], F32, tag="po",
                                        name=f"po{qb}{hp}{a}")
                            for a in range(2):
                                h = 2 * hp + a
                                nc.tensor.matmul(
                                    ps_os[a][:, col0:512],
                                    v_sb[:, kt_i * VW + h * (DH + 1):
                                         kt_i * VW + h * (DH + 1) + DH + 1],
                                    pt[:, a * 512 + col0:(a + 1) * 512],
                                    start=(kt_i == 0), stop=(kt_i == nk - 1),
                                    skip_group_check=True)

                        last_hp = (qb == NQB - 1 and hp == 1)
                        for kt_i in range(nk):
                            off = kt_i * 128 - q0
                            col0 = max(0, off) if causal else 0
                            ps_s = score_ps.tile([128, 1024], F32, tag="sc",
                                                 name=f"sc{qb}{hp}{kt_i}")
                            pt = pt_pool.tile([128, 1024], MM_DT, tag="pt",
                                              name=f"pt{qb}{hp}{kt_i}")
                            # the two heads' score matmuls target different PE
                            # row-groups (rows 0-63 vs 64-127)
                            for a in range(2):
                                p0 = a * 64
                                nc.tensor.matmul(
                                    ps_s[:, a * 512 + col0:(a + 1) * 512],
                                    kt_sb[p0:p0 + 64,
                                          t * S + kt_i * 128: t * S + kt_i * 128 + 128],
                                    qt_sb[p0:p0 + 64,
                                          t * S + q0 + col0: t * S + q0 + 512],
                                    start=True, stop=True)
                            if col0 == 0:
                                nc.scalar.activation(pt[:], ps_s[:], ActF.Exp,
                                                     scale=SCALE)
                            else:
                                pt3 = pt[:].rearrange(
                                    "p (u q) -> p u q", u=2)[:, :, col0:512]
                                ps3 = ps_s[:].rearrange(
                                    "p (u q) -> p u q", u=2)[:, :, col0:512]
                                nc.scalar.activation(pt3, ps3, ActF.Exp,
                                                     scale=SCALE)
                            if causal and off >= 0:
                                # triangular mask only on the 128-col diagonal
                                # chunk (cols < col0 are never read)
                                sel = pt[:].rearrange(
                                    "p (u q) -> p u q", u=2)[:, :, col0:col0 + 128]
                                nc.gpsimd.affine_select(
                                    out=sel, in_=sel,
                                    compare_op=Alu.is_ge,
                                    fill=0.0, base=0,
                                    pattern=[[0, 2], [1, 128]],
                                    channel_multiplier=-1)
                            if kt_i == 2:
                                # previous hp's norms land here, after its
                                # last deferred PVs have left the queue
                                # (and before any fillers: out-proj fillers
                                # read the ot these norms write)
                                flush_pending()
                            if kt_i >= 2:
                                flush_fillers(3 if len(pending_f) > 4 else
                                              2 if len(pending_f) > 2 else 1)
                            # defer this step's PV, across the hp boundary:
                            # the next hp's scores keep the PE fed while this
                            # hp's last exps drain on ACT
                            pv_queue.append(
                                lambda kt_i=kt_i, pt=pt, col0=col0,
                                f=emit_pv: f(kt_i, pt, col0))
                            pop_pv(2)

                        if qb == NQB - 1 and hp == 1:
                            # chunked so the first tail out-projections can
                            # start before the full norm finishes
                            pending.append(
                                lambda f=norm: f(chunks=((0, 128),
                                                         (128, 512))))
                        else:
                            pending.append(norm)
                        if hp == 0:
                            if causal and qb + 1 < NQB:
                                for st in range(4 * (qb + 1), 4 * (qb + 2)):
                                    pending_f.append(
                                        (("vp", st),
                                         lambda st=st: emit_vproj_filler(st)))
                            if qb == NQB - 1:
                                # cc0 half of the last q-block's out-proj:
                                # PE filler during its hp1 attention steps
                                for st in range(qb * 4, qb * 4 + 4):
                                    pending_f.append(
                                        (("opc0", st),
                                         lambda qb=qb, st=st:
                                         emit_outproj_st(qb, st, mode="cc0")))
                        else:
                            if qb < NQB - 1:
                                for st in range(qb * 4, qb * 4 + 4):
                                    pending_f.append(
                                        (("op", st),
                                         lambda qb=qb, st=st:
                                         emit_outproj_st(qb, st)))
                pop_pv(0)
                flush_pending()
                flush_fillers()
                for st in range((NQB - 1) * 4, NQB * 4):
                    emit_outproj_cc1_tail(st)


def _build(causal):
    nc = bacc.Bacc("TRN2", target_bir_lowering=False, debug=False,
                   num_devices=NCORES)
    with tile.TileContext(nc) as tc:
        _emit(nc, tc, causal)
    nc.compile()
    return nc


def _shard_inputs(QKV, Wq, bq, Wk, bk, Wv, bv, Wo, bo):
    QKV = np.asarray(QKV, dtype=np.float32)
    Wq, Wk, Wv, Wo = (np.asarray(w, dtype=np.float32) for w in (Wq, Wk, Wv, Wo))
    bq, bk, bv, bo = (np.asarray(b_, dtype=np.float32) for b_ in (bq, bk, bv, bo))
    in_maps = []
    for core in range(NCORES):
        b, g = divmod(core, TPW)
        cs = slice(g * C, (g + 1) * C)
        bqs, bks = bq[cs], bk[cs]
        bqk = np.stack([bqs[:128], bqs[128:], bks[:128], bks[128:]], axis=1)
        # x layout: [p, sb, ec, c] with x[p, sb, ec, c] = X[sb*512+c, ec*128+p]
        xt = np.ascontiguousarray(
            QKV[b].reshape(NQB, 512, NEC, 128).transpose(3, 0, 2, 1)
        ).reshape(128, -1).astype(MM_NP)
        # w layout: [p, ec, c] with w[p, ec, c] = W[g*C + t*128 + c, ec*128+p]
        def wlay(w, t):
            return np.ascontiguousarray(
                w[cs, :].T.reshape(NEC, 128, C)
                .transpose(1, 0, 2)[:, :, t * 128:(t + 1) * 128]
            ).reshape(128, -1).astype(MM_NP)
        # wo layout: [p, cc, e] = Wo[e, g*C + cc*128 + p]
        wot = np.ascontiguousarray(
            Wo[:, cs].T.reshape(2, 128, E).transpose(1, 0, 2)
        ).reshape(128, -1).astype(MM_NP)
        cstp = np.concatenate(
            [bqk, np.ones((128, NST * HPC), dtype=np.float32)], axis=1)
        cstr = np.concatenate(
            [bv[cs], (bo if g == 0 else np.zeros_like(bo)),
             np.ones(128, dtype=np.float32)]).reshape(1, -1)
        in_maps.append({
            "xt": xt,
            "wq0": wlay(Wq, 0),
            "wq1": wlay(Wq, 1),
            "wk0": wlay(Wk, 0),
            "wk1": wlay(Wk, 1),
            "wvt": np.ascontiguousarray(
                Wv[cs, :].T.reshape(NEC, 128, C).transpose(1, 0, 2)
            ).reshape(128, -1).astype(MM_NP),
            "wot": wot,
            "cstp": np.ascontiguousarray(cstp),
            "cstr": np.ascontiguousarray(cstr),
        })
    return in_maps


def kernel(QKV, Wq, bq, Wk, bk, Wv, bv, Wo, bo, is_causal):
    causal = bool(int(np.asarray(is_causal)))
    if causal not in _cache:
        _cache[causal] = _build(causal)
    nc = _cache[causal]
    in_maps = _shard_inputs(QKV, Wq, bq, Wk, bk, Wv, bv, Wo, bo)
    res = run_bass_kernel_spmd(nc, in_maps, core_ids=list(range(NCORES)))
    out = np.empty((B, S, E), dtype=np.float32)
    for b in range(B):
        acc = res.results[TPW * b]["out"].astype(np.float32)
        for g in range(1, TPW):
            acc = acc + res.results[TPW * b + g]["out"].astype(np.float32)
        out[b] = acc
    return out
